# revision 1
# baseline (speedup 1.0000x reference)
"""Trainium2 Bass kernel for causal GQA self-attention (B=2, S=2048, H=2048,
16 heads / 4 KV heads, head_dim 128) on 8 NeuronCores.

Sharding: 8-way head-tensor-parallel over the combined batch for QKV+attention
(core i owns heads {2i, 2i+1} and KV head i//2, full 4096 = B*S rows), then a
single 8-rank AllToAll switches to row-sharding so each core computes 512 rows
of the output projection with the full Wo. No all-reduce needed.

Per-core dataflow (all layouts chosen so no on-device transposes of the big
activations are ever needed; softmax runs over the PSUM partition dim via a
ones-vector matmul for the column sums):
  XT[h, r] (bf16, host-pretransposed) --matmul--> QT/KT/VT (channels on
  partitions), V via PE transpose of VT; scores S^T[k, q] = KT_tile.T @ QT;
  causal handled by compile-time tile skipping + one [128,128] triangle mask;
  exp on ScalarE (scale folded in); AV and column-sum accumulated in PSUM over
  k tiles; normalize by broadcasted reciprocal; AllToAll; o_proj with bias
  folded in as a rank-1 matmul.

The attention_mask input is all-ones for this problem (spec fill=ones), so it
is ignored. Matmuls run as float32r (full-rate fp32) except the QKV
projections, which use bf16 inputs.
"""

import sys

sys.path.insert(0, "/opt/trn_rl_repo")

from contextlib import ExitStack

import numpy as np
import ml_dtypes

import concourse.bass as bass
import concourse.mybir as mybir
import concourse.tile as tile
from concourse import bacc
from concourse.bass_utils import run_bass_kernel_spmd
from concourse.masks import make_identity

F32 = mybir.dt.float32
F32R = mybir.dt.float32r
BF16 = mybir.dt.bfloat16
AF = mybir.ActivationFunctionType

N_CORES = 8
B, S, HID = 2, 2048, 2048
NH, NKV, D = 16, 4, 128
R = B * S  # 4096 combined rows
SCALE = 1.0 / np.sqrt(D)
NEG = -1e30
P = 128
N_KT = HID // P  # 16 contraction tiles
N_RB = R // 512  # 8 row blocks


def build_nc(debug=False):
    nc = bacc.Bacc("TRN2", target_bir_lowering=False, debug=debug, num_devices=8)

    xt = nc.dram_tensor("xt", [HID, R], BF16, kind="ExternalInput")
    wq = nc.dram_tensor("wq", [HID, 256], BF16, kind="ExternalInput")
    wk = nc.dram_tensor("wk", [HID, 128], BF16, kind="ExternalInput")
    wv = nc.dram_tensor("wv", [HID, 128], BF16, kind="ExternalInput")
    bq = nc.dram_tensor("bq", [256, 1], F32, kind="ExternalInput")
    bk = nc.dram_tensor("bk", [128, 1], F32, kind="ExternalInput")
    bv = nc.dram_tensor("bv", [128, 1], F32, kind="ExternalInput")
    wo = nc.dram_tensor("wo", [HID, HID], BF16, kind="ExternalInput")
    bo = nc.dram_tensor("bo", [1, HID], BF16, kind="ExternalInput")
    mtri = nc.dram_tensor("mtri", [P, P], F32, kind="ExternalInput")
    onesd = nc.dram_tensor("onesd", [P, P], BF16, kind="ExternalInput")
    y = nc.dram_tensor("y", [512, HID], F32, kind="ExternalOutput")

    with tile.TileContext(nc) as tc, ExitStack() as top:
        persist = top.enter_context(tc.tile_pool(name="persist", bufs=1))
        dram = top.enter_context(tc.tile_pool(name="dram", bufs=1, space="DRAM"))

        a2a_in = [dram.tile([8, P, 512], BF16, name=f"a2a_in{h}") for h in range(2)]
        a2a_out = [dram.tile([8, P, 512], BF16, name=f"a2a_out{h}") for h in range(2)]

        # QKV weights first: they gate the very first matmuls. Small consts
        # ride the gpsimd queue so they can't delay the weight/XT stream.
        wq_sb = persist.tile([P, N_KT, 256], BF16, tag="wq")
        nc.sync.dma_start(wq_sb[:], wq[:].rearrange("(t p) c -> p t c", p=P))
        wk_sb = persist.tile([P, N_KT, 128], BF16, tag="wk")
        nc.sync.dma_start(wk_sb[:], wk[:].rearrange("(t p) c -> p t c", p=P))
        wv_sb = persist.tile([P, N_KT, 128], BF16, tag="wv")
        nc.sync.dma_start(wv_sb[:], wv[:].rearrange("(t p) c -> p t c", p=P))

        ident = persist.tile([P, P], F32, tag="ident")
        make_identity(nc, ident)
        ones_sq = persist.tile([P, P], BF16, tag="ones_sq")
        nc.gpsimd.dma_start(ones_sq[:], onesd[:])
        mtri_sb = persist.tile([P, P], F32, tag="mtri")
        nc.gpsimd.dma_start(mtri_sb[:], mtri[:])
        bq_sb = persist.tile([P, 2], F32, tag="bq")
        nc.gpsimd.dma_start(bq_sb[:, 0:1], bq[0:128, :])
        nc.gpsimd.dma_start(bq_sb[:, 1:2], bq[128:256, :])
        bk_sb = persist.tile([P, 1], F32, tag="bk")
        nc.gpsimd.dma_start(bk_sb[:], bk[:])
        bv_sb = persist.tile([P, 1], F32, tag="bv")
        nc.gpsimd.dma_start(bv_sb[:], bv[:])
        bo_sb = persist.tile([1, HID], BF16, tag="bo")
        nc.gpsimd.dma_start(bo_sb[:], bo[:])

        # channel-major activations: partitions = feature dim
        qt_sb = [persist.tile([P, R], BF16, tag=f"qt{c}", name=f"qt{c}") for c in range(2)]
        kt_sb = persist.tile([P, R], BF16, tag="kt")
        vt_sb = persist.tile([P, R], F32, tag="vt")
        v_sb = persist.tile([P, R // P, P], BF16, tag="v")  # [krow%128, ktile, d]

        xt_r = xt[:].rearrange("(t p) r -> p t r", p=P)

        # ---- Phase 1: QKV projections (+ V transpose) ----
        with ExitStack() as ph1:
            xpool = ph1.enter_context(tc.tile_pool(name="xp", bufs=2))
            pspool = ph1.enter_context(tc.tile_pool(name="ps1", bufs=5, space="PSUM"))
            ptpool = ph1.enter_context(tc.tile_pool(name="pst", bufs=2, space="PSUM"))
            for rb in range(N_RB):
                rsl = slice(512 * rb, 512 * (rb + 1))
                xt_t = xpool.tile([P, N_KT, 512], BF16, tag="x", name="xt_t")
                for kc in range(4):  # 4 chunks so matmuls start on first arrival
                    nc.sync.dma_start(
                        xt_t[:, 4 * kc : 4 * (kc + 1), :],
                        xt_r[:, 4 * kc : 4 * (kc + 1), rsl],
                    )
                ps_q0 = pspool.tile([P, 512], F32, tag="ps1", name="ps_q0")
                ps_q1 = pspool.tile([P, 512], F32, tag="ps1", name="ps_q1")
                ps_k = pspool.tile([P, 512], F32, tag="ps1", name="ps_k")
                ps_v = pspool.tile([P, 512], F32, tag="ps1", name="ps_v")
                for kt_i in range(N_KT):
                    st, sp = kt_i == 0, kt_i == N_KT - 1
                    x_sl = xt_t[:, kt_i, :]
                    nc.tensor.matmul(ps_q0[:], wq_sb[:, kt_i, 0:128], x_sl, start=st, stop=sp)
                    nc.tensor.matmul(ps_q1[:], wq_sb[:, kt_i, 128:256], x_sl, start=st, stop=sp)
                    nc.tensor.matmul(ps_k[:], wk_sb[:, kt_i, :], x_sl, start=st, stop=sp)
                    nc.tensor.matmul(ps_v[:], wv_sb[:, kt_i, :], x_sl, start=st, stop=sp)
                nc.scalar.activation(qt_sb[0][:, rsl], ps_q0[:], AF.Identity, bias=bq_sb[:, 0:1])
                nc.scalar.activation(qt_sb[1][:, rsl], ps_q1[:], AF.Identity, bias=bq_sb[:, 1:2])
                nc.scalar.activation(kt_sb[:, rsl], ps_k[:], AF.Identity, bias=bk_sb[:])
                nc.scalar.activation(vt_sb[:, rsl], ps_v[:], AF.Identity, bias=bv_sb[:])
                for j in range(4):
                    m = 4 * rb + j
                    ps_t = ptpool.tile([P, P], F32, tag="pt", name="ps_t")
                    nc.tensor.transpose(ps_t[:], vt_sb[:, P * m : P * (m + 1)], ident[:])
                    nc.vector.tensor_copy(v_sb[:, m, :], ps_t[:])

        # ---- Phase 2: attention (flash-style, S^T layout) ----
        # h outer so each head's A2A half can fire as soon as that head is
        # done on every (b, qb); the collective then overlaps remaining work.
        with ExitStack() as ph2:
            espool = ph2.enter_context(tc.tile_pool(name="es", bufs=4))
            bcpool = ph2.enter_context(tc.tile_pool(name="bc", bufs=2))
            aopool = ph2.enter_context(tc.tile_pool(name="ao", bufs=2))
            rcpool = ph2.enter_context(tc.tile_pool(name="rc", bufs=2))
            pss = ph2.enter_context(tc.tile_pool(name="pss", bufs=4, space="PSUM"))
            psav = ph2.enter_context(tc.tile_pool(name="psav", bufs=2, space="PSUM"))
            pscs = ph2.enter_context(tc.tile_pool(name="pscs", bufs=2, space="PSUM"))
            for h in range(2):
                for b in range(B):
                    for qb in range(4):
                        # diagonal k-tiles first (full q width on the first)
                        ktiles = list(range(4 * qb, 4 * qb + 4)) + list(range(4 * qb))
                        ps_av = psav.tile([P, 512], F32, tag="av", name="ps_av")
                        ps_cs = pscs.tile([1, 512], F32, tag="cs", name="ps_cs")
                        n_kt_q = len(ktiles)

                        def emit_av(ki, q0, es, st, sp):
                            nc.tensor.matmul(
                                ps_av[:, q0:512], v_sb[:, 16 * b + ki, :],
                                es[:, q0:512], start=st, stop=sp,
                                skip_group_check=True,
                            )
                            nc.tensor.matmul(
                                ps_cs[:, q0:512], ones_sq[:, 0:1],
                                es[:, q0:512], start=st, stop=sp,
                                skip_group_check=True,
                            )

                        pending = None  # software-pipeline AV one k-tile behind
                        for idx, ki in enumerate(ktiles):
                            diag = ki >= 4 * qb
                            q0 = 128 * ki - 512 * qb if diag else 0
                            ps_s = pss.tile([P, 512], F32, tag="s", name="ps_s")
                            ksl = kt_sb[:, S * b + P * ki : S * b + P * (ki + 1)]
                            qsl = qt_sb[h][:, S * b + 512 * qb + q0 : S * b + 512 * (qb + 1)]
                            nc.tensor.matmul(
                                ps_s[:, q0:512], ksl, qsl,
                                start=True, stop=True,
                            )
                            if diag:
                                nc.vector.tensor_add(
                                    ps_s[:, q0 : q0 + P], ps_s[:, q0 : q0 + P], mtri_sb[:]
                                )
                            es = espool.tile([P, 512], BF16, tag="es", name="es")
                            nc.scalar.activation(
                                es[:, q0:512], ps_s[:, q0:512], AF.Exp, scale=SCALE
                            )
                            if pending is not None:
                                emit_av(*pending)
                            pending = (ki, q0, es, idx == 0, idx == n_kt_q - 1)
                        emit_av(*pending)

                        recip = rcpool.tile([1, 512], F32, tag="rc", name="recip")
                        nc.vector.reciprocal(recip[:], ps_cs[:])
                        bc = bcpool.tile([P, 512], F32, tag="bc", name="bc")
                        nc.gpsimd.partition_broadcast(bc[:], recip[:])
                        ao = aopool.tile([P, 512], BF16, tag="ao", name="ao")
                        nc.vector.tensor_mul(ao[:], ps_av[:], bc[:])
                        nc.sync.dma_start(a2a_in[h][4 * b + qb, :, :], ao[:])

                # ---- AllToAll for this head half (overlaps remaining work)
                nc.gpsimd.collective_compute(
                    "AllToAll",
                    mybir.AluOpType.bypass,
                    replica_groups=[list(range(N_CORES))],
                    ins=[a2a_in[h][:]],
                    outs=[a2a_out[h][:]],
                )

        # ---- Phase 4: o_proj (512 rows x 2048, full Wo) ----
        # even hd-tiles (head 0 of each peer) arrive with the first A2A; the
        # odd-tile accumulation and bias ride behind the second one.
        with ExitStack() as ph4:
            atpool = ph4.enter_context(tc.tile_pool(name="at", bufs=1))
            wopool = ph4.enter_context(tc.tile_pool(name="wop", bufs=8))
            ypool = ph4.enter_context(tc.tile_pool(name="yp", bufs=4))
            pso = ph4.enter_context(tc.tile_pool(name="pso", bufs=8, space="PSUM"))
            at = [None] * N_KT
            # All at-tile loads go on the gpsimd (SWDGE) queue, evens first:
            # the odd tiles wait on the second AllToAll, and on the sync queue
            # they would head-of-line-block the Wo stream that the even-tile
            # accumulation pass needs.
            for t in list(range(0, N_KT, 2)) + list(range(1, N_KT, 2)):
                a = atpool.tile([P, 512], BF16, tag=f"at{t}", name=f"at{t}")
                nc.gpsimd.dma_start(a[:], a2a_out[t % 2][t // 2, :, :])
                at[t] = a
            # pass 1: even hd-tiles (ready after the first A2A) + bias -> y.
            # Runs over all nb while the second A2A is still in flight.
            for nb in range(4):
                nsl = slice(512 * nb, 512 * (nb + 1))
                ps_os = [pso.tile([P, 512], F32, tag="po", name=f"ps_o{q}") for q in range(4)]
                for ti, t in enumerate(range(0, N_KT, 2)):
                    wo_t = wopool.tile([P, 512], BF16, tag="wo", name="wo_t")
                    nc.sync.dma_start(wo_t[:], wo[P * t : P * (t + 1), nsl])
                    for qt_i in range(4):
                        nc.tensor.matmul(
                            ps_os[qt_i][:], at[t][:, P * qt_i : P * (qt_i + 1)],
                            wo_t[:], start=(ti == 0), stop=False,
                            skip_group_check=True,
                        )
                for qt_i in range(4):
                    nc.tensor.matmul(
                        ps_os[qt_i][:], ones_sq[0:1, :],
                        bo_sb[0:1, nsl], start=False, stop=True,
                        skip_group_check=True,
                    )
                    ysb = ypool.tile([P, 512], F32, tag="y", name="ysb")
                    nc.scalar.activation(ysb[:], ps_os[qt_i][:], AF.Copy)
                    nc.sync.dma_start(y[P * qt_i : P * (qt_i + 1), nsl], ysb[:])
            # pass 2: odd hd-tiles (behind the second A2A), CCE-accumulated
            # into y so no PSUM bank is held across the A2A wait.
            for nb in range(4):
                nsl = slice(512 * nb, 512 * (nb + 1))
                ps_o2 = [pso.tile([P, 512], F32, tag="po", name=f"ps_p{q}") for q in range(4)]
                for ti, t in enumerate(range(1, N_KT, 2)):
                    wo_t = wopool.tile([P, 512], BF16, tag="wo", name="wo_t")
                    nc.sync.dma_start(wo_t[:], wo[P * t : P * (t + 1), nsl])
                    for qt_i in range(4):
                        nc.tensor.matmul(
                            ps_o2[qt_i][:], at[t][:, P * qt_i : P * (qt_i + 1)],
                            wo_t[:], start=(ti == 0), stop=(ti == N_KT // 2 - 1),
                            skip_group_check=True,
                        )
                for qt_i in range(4):
                    ysb = ypool.tile([P, 512], F32, tag="y", name="ysb")
                    nc.scalar.activation(ysb[:], ps_o2[qt_i][:], AF.Copy)
                    nc.gpsimd.dma_start(
                        y[P * qt_i : P * (qt_i + 1), nsl], ysb[:],
                        accum_op=mybir.AluOpType.add,
                    )

    nc.compile()
    return nc


def make_in_maps(hidden_states, Wq, bq, Wk, bk, Wv, bv, Wo, bo):
    X = np.asarray(hidden_states, np.float32).reshape(R, HID)
    XT = np.ascontiguousarray(X.T).astype(ml_dtypes.bfloat16)
    qq = np.arange(P)[None, :]
    kk = np.arange(P)[:, None]
    mtri = np.where(qq >= kk, 0.0, NEG).astype(np.float32)
    Wq = np.asarray(Wq, np.float32)
    Wk = np.asarray(Wk, np.float32)
    Wv = np.asarray(Wv, np.float32)
    Wo = np.ascontiguousarray(np.asarray(Wo, np.float32)).astype(ml_dtypes.bfloat16)
    bq = np.asarray(bq, np.float32)
    bk = np.asarray(bk, np.float32)
    bv = np.asarray(bv, np.float32)
    bo = np.asarray(bo, np.float32)
    in_maps = []
    for i in range(N_CORES):
        g = i // 2
        in_maps.append({
            "xt": XT,
            "wq": np.ascontiguousarray(Wq[:, 256 * i : 256 * (i + 1)]).astype(ml_dtypes.bfloat16),
            "wk": np.ascontiguousarray(Wk[:, 128 * g : 128 * (g + 1)]).astype(ml_dtypes.bfloat16),
            "wv": np.ascontiguousarray(Wv[:, 128 * g : 128 * (g + 1)]).astype(ml_dtypes.bfloat16),
            "bq": np.ascontiguousarray(bq[256 * i : 256 * (i + 1)]).reshape(256, 1),
            "bk": np.ascontiguousarray(bk[128 * g : 128 * (g + 1)]).reshape(128, 1),
            "bv": np.ascontiguousarray(bv[128 * g : 128 * (g + 1)]).reshape(128, 1),
            "wo": Wo,
            "bo": bo.reshape(1, HID).astype(ml_dtypes.bfloat16),
            "mtri": mtri,
            "onesd": np.ones((P, P), ml_dtypes.bfloat16),
        })
    return in_maps


def assemble(results):
    Y = np.empty((B, S, HID), np.float32)
    for i in range(N_CORES):
        b, c = i // 4, i % 4
        Y[b, 512 * c : 512 * (c + 1), :] = results[i]["y"]
    return Y


_NC_CACHE = {}


def _get_nc(debug=False):
    if debug not in _NC_CACHE:
        _NC_CACHE[debug] = build_nc(debug=debug)
    return _NC_CACHE[debug]


def kernel(hidden_states, attention_mask, Wq, bq, Wk, bk, Wv, bv, Wo, bo):
    # attention_mask is all-ones for this problem (spec: fill=ones) -> ignored
    nc = _get_nc(debug=False)
    in_maps = make_in_maps(hidden_states, Wq, bq, Wk, bk, Wv, bv, Wo, bo)
    res = run_bass_kernel_spmd(nc, in_maps, core_ids=list(range(N_CORES)))
    return assemble(res.results)



# revision 5
# speedup vs baseline: 1.2231x; 1.2231x over previous
"""Trainium2 Bass kernel for causal GQA self-attention (B=2, S=2048, H=2048,
16 heads / 4 KV heads, head_dim 128) on 8 NeuronCores.

Sharding: 8-way head-tensor-parallel over the combined batch for QKV+attention
(core i owns heads {2i, 2i+1} and KV head i//2, full 4096 = B*S rows), then a
single 8-rank AllToAll switches to row-sharding so each core computes 512 rows
of the output projection with the full Wo. No all-reduce needed.

Per-core dataflow (all layouts chosen so no on-device transposes of the big
activations are ever needed; softmax runs over the PSUM partition dim via a
ones-matrix matmul whose [128,512] output IS the broadcast of the column sums):
  XT[h, r] (bf16, host-pretransposed) --matmul--> QT/KT/VT (channels on
  partitions), V via PE transpose of VT; scores S^T[k, q] = KT_tile.T @ QT;
  causal handled by compile-time tile skipping + one [128,128] triangle mask
  added on GpSimd; exp on ScalarE (scale folded in); AV and 128-row column-sum
  accumulated in PSUM over k tiles; normalize with reciprocal_approx_fast +
  one vector multiply; AllToAll; o_proj from SBUF-resident Wo with bias folded
  in as a rank-1 matmul.

The attention_mask input is all-ones for this problem (spec fill=ones), so it
is ignored. All matmuls take bf16 inputs (fp32 PSUM accumulate).
"""

import sys

sys.path.insert(0, "/opt/trn_rl_repo")

from contextlib import ExitStack

import numpy as np
import ml_dtypes

import concourse.bass as bass
import concourse.mybir as mybir
import concourse.tile as tile
from concourse import bacc
from concourse.bass_utils import run_bass_kernel_spmd
from concourse.masks import make_identity

F32 = mybir.dt.float32
BF16 = mybir.dt.bfloat16
AF = mybir.ActivationFunctionType

N_CORES = 8
B, S, HID = 2, 2048, 2048
NH, NKV, D = 16, 4, 128
R = B * S  # 4096 combined rows
SCALE = 1.0 / np.sqrt(D)
NEG = -1e30
P = 128
N_KT = HID // P  # 16 contraction tiles
N_RB = R // 512  # 8 row blocks


def build_nc(debug=False):
    nc = bacc.Bacc("TRN2", target_bir_lowering=False, debug=debug, num_devices=8)

    xt = nc.dram_tensor("xt", [HID, R], BF16, kind="ExternalInput")
    wq = nc.dram_tensor("wq", [HID, 256], BF16, kind="ExternalInput")
    wk = nc.dram_tensor("wk", [HID, 128], BF16, kind="ExternalInput")
    wv = nc.dram_tensor("wv", [HID, 128], BF16, kind="ExternalInput")
    bq = nc.dram_tensor("bq", [256, 1], F32, kind="ExternalInput")
    bk = nc.dram_tensor("bk", [128, 1], F32, kind="ExternalInput")
    bv = nc.dram_tensor("bv", [128, 1], F32, kind="ExternalInput")
    wo = nc.dram_tensor("wo", [HID, HID], BF16, kind="ExternalInput")
    bo = nc.dram_tensor("bo", [1, HID], BF16, kind="ExternalInput")
    mtri = nc.dram_tensor("mtri", [P, P], BF16, kind="ExternalInput")
    onesd = nc.dram_tensor("onesd", [P, P], BF16, kind="ExternalInput")
    y = nc.dram_tensor("y", [512, HID], F32, kind="ExternalOutput")

    with tile.TileContext(nc) as tc, ExitStack() as top:
        persist = top.enter_context(tc.tile_pool(name="persist", bufs=1))
        dram = top.enter_context(tc.tile_pool(name="dram", bufs=1, space="DRAM"))

        a2a_in = [dram.tile([8, P, 512], BF16, name=f"a2a_in{h}") for h in range(2)]
        a2a_out = [dram.tile([8, P, 512], BF16, name=f"a2a_out{h}") for h in range(2)]

        # QKV weights + first XT block gate the very first matmuls: interleave
        # their chunks on the sync queue so matmuls start after ~1MB of DMA.
        wq_sb = persist.tile([P, N_KT, 256], BF16, tag="wq")
        wk_sb = persist.tile([P, N_KT, 128], BF16, tag="wk")
        wv_sb = persist.tile([P, N_KT, 128], BF16, tag="wv")
        wq_r = wq[:].rearrange("(t p) c -> p t c", p=P)
        wk_r = wk[:].rearrange("(t p) c -> p t c", p=P)
        wv_r = wv[:].rearrange("(t p) c -> p t c", p=P)
        nc.sync.dma_start(wq_sb[:, 0:4, :], wq_r[:, 0:4, :])
        nc.sync.dma_start(wk_sb[:, 0:4, :], wk_r[:, 0:4, :])
        nc.sync.dma_start(wv_sb[:, 0:4, :], wv_r[:, 0:4, :])

        xt_r = xt[:].rearrange("(t p) r -> p t r", p=P)

        # Small consts ride the gpsimd queue; the SBUF-resident Wo streams in
        # behind them (it is only needed in phase 4, ~300us later).
        ident = persist.tile([P, P], BF16, tag="ident")
        make_identity(nc, ident)
        ones_sq = persist.tile([P, P], BF16, tag="ones_sq")
        nc.gpsimd.dma_start(ones_sq[:], onesd[:])
        mtri_sb = persist.tile([P, P], BF16, tag="mtri")
        nc.gpsimd.dma_start(mtri_sb[:], mtri[:])
        bq_sb = persist.tile([P, 2], F32, tag="bq")
        nc.gpsimd.dma_start(bq_sb[:, 0:1], bq[0:128, :])
        nc.gpsimd.dma_start(bq_sb[:, 1:2], bq[128:256, :])
        bk_sb = persist.tile([P, 1], F32, tag="bk")
        nc.gpsimd.dma_start(bk_sb[:], bk[:])
        bv_sb = persist.tile([P, 1], F32, tag="bv")
        nc.gpsimd.dma_start(bv_sb[:], bv[:])
        bo_sb = persist.tile([1, HID], BF16, tag="bo")
        nc.gpsimd.dma_start(bo_sb[:], bo[:])
        wo_sb = persist.tile([P, N_KT, HID], BF16, tag="wo")
        for t in range(N_KT):
            nc.gpsimd.dma_start(wo_sb[:, t, :], wo[P * t : P * (t + 1), :])

        # channel-major activations: partitions = feature dim
        qt_sb = [persist.tile([P, R], BF16, tag=f"qt{c}", name=f"qt{c}") for c in range(2)]
        kt_sb = persist.tile([P, R], BF16, tag="kt")
        vt_sb = persist.tile([P, R], BF16, tag="vt")
        v_sb = persist.tile([P, R // P, P], BF16, tag="v")  # [krow%128, ktile, d]

        # ---- Phase 1: QKV projections (+ V transpose) ----
        with ExitStack() as ph1:
            xpool = ph1.enter_context(tc.tile_pool(name="xp", bufs=2))
            pspool = ph1.enter_context(tc.tile_pool(name="ps1", bufs=5, space="PSUM"))
            ptpool = ph1.enter_context(tc.tile_pool(name="pst", bufs=2, space="PSUM"))
            for rb in range(N_RB):
                rsl = slice(512 * rb, 512 * (rb + 1))
                xt_t = xpool.tile([P, N_KT, 512], BF16, tag="x", name="xt_t")
                for kc in range(4):  # 4 chunks so matmuls start on first arrival
                    nc.sync.dma_start(
                        xt_t[:, 4 * kc : 4 * (kc + 1), :],
                        xt_r[:, 4 * kc : 4 * (kc + 1), rsl],
                    )
                    if rb == 0 and kc < 3:
                        # stream the remaining weight chunks behind the first
                        # xt chunks (needed from kt=4 onwards)
                        c = kc + 1
                        nc.sync.dma_start(wq_sb[:, 4 * c : 4 * c + 4, :], wq_r[:, 4 * c : 4 * c + 4, :])
                        nc.sync.dma_start(wk_sb[:, 4 * c : 4 * c + 4, :], wk_r[:, 4 * c : 4 * c + 4, :])
                        nc.sync.dma_start(wv_sb[:, 4 * c : 4 * c + 4, :], wv_r[:, 4 * c : 4 * c + 4, :])
                ps_q0 = pspool.tile([P, 512], F32, tag="ps1", name="ps_q0")
                ps_q1 = pspool.tile([P, 512], F32, tag="ps1", name="ps_q1")
                ps_k = pspool.tile([P, 512], F32, tag="ps1", name="ps_k")
                ps_v = pspool.tile([P, 512], F32, tag="ps1", name="ps_v")
                for kt_i in range(N_KT):
                    st, sp = kt_i == 0, kt_i == N_KT - 1
                    x_sl = xt_t[:, kt_i, :]
                    nc.tensor.matmul(ps_q0[:], wq_sb[:, kt_i, 0:128], x_sl, start=st, stop=sp)
                    nc.tensor.matmul(ps_q1[:], wq_sb[:, kt_i, 128:256], x_sl, start=st, stop=sp)
                    nc.tensor.matmul(ps_k[:], wk_sb[:, kt_i, :], x_sl, start=st, stop=sp)
                    nc.tensor.matmul(ps_v[:], wv_sb[:, kt_i, :], x_sl, start=st, stop=sp)
                nc.vector.tensor_scalar_add(qt_sb[0][:, rsl], ps_q0[:], bq_sb[:, 0:1])
                nc.vector.tensor_scalar_add(qt_sb[1][:, rsl], ps_q1[:], bq_sb[:, 1:2])
                nc.vector.tensor_scalar_add(kt_sb[:, rsl], ps_k[:], bk_sb[:])
                nc.vector.tensor_scalar_add(vt_sb[:, rsl], ps_v[:], bv_sb[:])
                for j in range(4):
                    m = 4 * rb + j
                    ps_t = ptpool.tile([P, P], BF16, tag="pt", name="ps_t")
                    nc.tensor.transpose(ps_t[:], vt_sb[:, P * m : P * (m + 1)], ident[:])
                    nc.vector.tensor_copy(v_sb[:, m, :], ps_t[:])

        # ---- Phase 2: attention (flash-style, S^T layout) ----
        # h outer so each head's A2A half can fire as soon as that head is
        # done on every (b, qb); the collective then overlaps remaining work.
        with ExitStack() as ph2:
            espool = ph2.enter_context(tc.tile_pool(name="es", bufs=6))
            bcpool = ph2.enter_context(tc.tile_pool(name="bc", bufs=2))
            aopool = ph2.enter_context(tc.tile_pool(name="ao", bufs=2))
            pss = ph2.enter_context(tc.tile_pool(name="pss", bufs=4, space="PSUM"))
            psav = ph2.enter_context(tc.tile_pool(name="psav", bufs=2, space="PSUM"))
            pscs = ph2.enter_context(tc.tile_pool(name="pscs", bufs=2, space="PSUM"))
            for h in range(2):
                for b in range(B):
                    for qb in range(4):
                        # diagonal k-tiles first (full q width on the first)
                        ktiles = list(range(4 * qb, 4 * qb + 4)) + list(range(4 * qb))
                        ps_av = psav.tile([P, 512], F32, tag="av", name="ps_av")
                        ps_cs = pscs.tile([P, 512], F32, tag="cs", name="ps_cs")
                        n_kt_q = len(ktiles)

                        def emit_av(ki, q0, es, st, sp):
                            nc.tensor.matmul(
                                ps_av[:, q0:512], v_sb[:, 16 * b + ki, :],
                                es[:, q0:512], start=st, stop=sp,
                                skip_group_check=True,
                            )
                            nc.tensor.matmul(
                                ps_cs[:, q0:512], ones_sq[:, :],
                                es[:, q0:512], start=st, stop=sp,
                                skip_group_check=True,
                            )

                        pending = []  # software-pipeline AV two k-tiles behind
                        for idx, ki in enumerate(ktiles):
                            diag = ki >= 4 * qb
                            q0 = 128 * ki - 512 * qb if diag else 0
                            ps_s = pss.tile([P, 512], F32, tag="s", name="ps_s")
                            ksl = kt_sb[:, S * b + P * ki : S * b + P * (ki + 1)]
                            qsl = qt_sb[h][:, S * b + 512 * qb + q0 : S * b + 512 * (qb + 1)]
                            nc.tensor.matmul(
                                ps_s[:, q0:512], ksl, qsl,
                                start=True, stop=True,
                            )
                            es = espool.tile([P, 512], BF16, tag="es", name="es")
                            nc.scalar.activation(
                                es[:, q0:512], ps_s[:, q0:512], AF.Exp, scale=SCALE
                            )
                            if diag:
                                nc.gpsimd.tensor_mul(
                                    es[:, q0 : q0 + P], es[:, q0 : q0 + P], mtri_sb[:]
                                )
                            if len(pending) == 2:
                                emit_av(*pending.pop(0))
                            pending.append((ki, q0, es, idx == 0, idx == n_kt_q - 1))
                        for args in pending:
                            emit_av(*args)

                        bc = bcpool.tile([P, 512], F32, tag="bc", name="bc")
                        nc.vector.reciprocal_approx_fast(out=bc[:], in_=ps_cs[:])
                        ao = aopool.tile([P, 512], BF16, tag="ao", name="ao")
                        nc.vector.tensor_mul(ao[:], ps_av[:], bc[:])
                        nc.sync.dma_start(a2a_in[h][4 * b + qb, :, :], ao[:])

                # ---- AllToAll for this head half (overlaps remaining work)
                nc.gpsimd.collective_compute(
                    "AllToAll",
                    mybir.AluOpType.bypass,
                    replica_groups=[list(range(N_CORES))],
                    ins=[a2a_in[h][:]],
                    outs=[a2a_out[h][:]],
                )

        # ---- Phase 4: o_proj (512 rows x 2048, SBUF-resident Wo) ----
        # even hd-tiles (head 0 of each peer) arrive with the first A2A; the
        # odd-tile accumulation and bias ride behind the second one.
        with ExitStack() as ph4:
            atpool = ph4.enter_context(tc.tile_pool(name="at", bufs=1))
            ypool = ph4.enter_context(tc.tile_pool(name="yp", bufs=4))
            pso = ph4.enter_context(tc.tile_pool(name="pso", bufs=8, space="PSUM"))
            at = [None] * N_KT
            # All at-tile loads go on the gpsimd (SWDGE) queue, evens first:
            # the odd tiles wait on the second AllToAll, and on the sync queue
            # they would head-of-line-block anything behind them.
            for t in list(range(0, N_KT, 2)) + list(range(1, N_KT, 2)):
                a = atpool.tile([P, 512], BF16, tag=f"at{t}", name=f"at{t}")
                nc.gpsimd.dma_start(a[:], a2a_out[t % 2][t // 2, :, :])
                at[t] = a
            # pass 1: even hd-tiles (ready after the first A2A) + bias -> y.
            # Runs over all nb while the second A2A is still in flight.
            for nb in range(4):
                nsl = slice(512 * nb, 512 * (nb + 1))
                ps_os = [pso.tile([P, 512], F32, tag="po", name=f"ps_o{q}") for q in range(4)]
                for ti, t in enumerate(range(0, N_KT, 2)):
                    for qt_i in range(4):
                        nc.tensor.matmul(
                            ps_os[qt_i][:], at[t][:, P * qt_i : P * (qt_i + 1)],
                            wo_sb[:, t, nsl], start=(ti == 0), stop=False,
                            skip_group_check=True,
                        )
                for qt_i in range(4):
                    nc.tensor.matmul(
                        ps_os[qt_i][:], ones_sq[0:1, :],
                        bo_sb[0:1, nsl], start=False, stop=True,
                        skip_group_check=True,
                    )
                    ysb = ypool.tile([P, 512], F32, tag="y", name="ysb")
                    nc.vector.tensor_copy(ysb[:], ps_os[qt_i][:])
                    nc.sync.dma_start(y[P * qt_i : P * (qt_i + 1), nsl], ysb[:])
            # pass 2: odd hd-tiles (behind the second A2A), CCE-accumulated
            # into y so no PSUM bank is held across the A2A wait.
            for nb in range(4):
                nsl = slice(512 * nb, 512 * (nb + 1))
                ps_o2 = [pso.tile([P, 512], F32, tag="po", name=f"ps_p{q}") for q in range(4)]
                for ti, t in enumerate(range(1, N_KT, 2)):
                    for qt_i in range(4):
                        nc.tensor.matmul(
                            ps_o2[qt_i][:], at[t][:, P * qt_i : P * (qt_i + 1)],
                            wo_sb[:, t, nsl], start=(ti == 0), stop=(ti == N_KT // 2 - 1),
                            skip_group_check=True,
                        )
                for qt_i in range(4):
                    ysb = ypool.tile([P, 512], F32, tag="y", name="ysb")
                    nc.vector.tensor_copy(ysb[:], ps_o2[qt_i][:])
                    nc.gpsimd.dma_start(
                        y[P * qt_i : P * (qt_i + 1), nsl], ysb[:],
                        accum_op=mybir.AluOpType.add,
                    )

    nc.compile()
    return nc


def make_in_maps(hidden_states, Wq, bq, Wk, bk, Wv, bv, Wo, bo):
    X = np.asarray(hidden_states, np.float32).reshape(R, HID)
    XT = np.ascontiguousarray(X.T).astype(ml_dtypes.bfloat16)
    qq = np.arange(P)[None, :]
    kk = np.arange(P)[:, None]
    mtri = np.where(qq >= kk, 1.0, 0.0).astype(ml_dtypes.bfloat16)
    Wq = np.asarray(Wq, np.float32)
    Wk = np.asarray(Wk, np.float32)
    Wv = np.asarray(Wv, np.float32)
    Wo = np.ascontiguousarray(np.asarray(Wo, np.float32)).astype(ml_dtypes.bfloat16)
    bq = np.asarray(bq, np.float32)
    bk = np.asarray(bk, np.float32)
    bv = np.asarray(bv, np.float32)
    bo = np.asarray(bo, np.float32)
    in_maps = []
    for i in range(N_CORES):
        g = i // 2
        in_maps.append({
            "xt": XT,
            "wq": np.ascontiguousarray(Wq[:, 256 * i : 256 * (i + 1)]).astype(ml_dtypes.bfloat16),
            "wk": np.ascontiguousarray(Wk[:, 128 * g : 128 * (g + 1)]).astype(ml_dtypes.bfloat16),
            "wv": np.ascontiguousarray(Wv[:, 128 * g : 128 * (g + 1)]).astype(ml_dtypes.bfloat16),
            "bq": np.ascontiguousarray(bq[256 * i : 256 * (i + 1)]).reshape(256, 1),
            "bk": np.ascontiguousarray(bk[128 * g : 128 * (g + 1)]).reshape(128, 1),
            "bv": np.ascontiguousarray(bv[128 * g : 128 * (g + 1)]).reshape(128, 1),
            "wo": Wo,
            "bo": bo.reshape(1, HID).astype(ml_dtypes.bfloat16),
            "mtri": mtri,
            "onesd": np.ones((P, P), ml_dtypes.bfloat16),
        })
    return in_maps


def assemble(results):
    Y = np.empty((B, S, HID), np.float32)
    for i in range(N_CORES):
        b, c = i // 4, i % 4
        Y[b, 512 * c : 512 * (c + 1), :] = results[i]["y"]
    return Y


_NC_CACHE = {}


def _get_nc(debug=False):
    if debug not in _NC_CACHE:
        _NC_CACHE[debug] = build_nc(debug=debug)
    return _NC_CACHE[debug]


def kernel(hidden_states, attention_mask, Wq, bq, Wk, bk, Wv, bv, Wo, bo):
    # attention_mask is all-ones for this problem (spec: fill=ones) -> ignored
    nc = _get_nc(debug=False)
    in_maps = make_in_maps(hidden_states, Wq, bq, Wk, bk, Wv, bv, Wo, bo)
    res = run_bass_kernel_spmd(nc, in_maps, core_ids=list(range(N_CORES)))
    return assemble(res.results)
